# revision 83
# baseline (speedup 1.0000x reference)
"""Augmented Chamfer distance on 8 Trainium2 NeuronCores.

Problem: x, y: [B=4, N=4096, 3] fp32.
  d2[b, n, m] = ||x[b,n] - y[b,m]||^2
  out = max( mean_{b,n} min_m d2,  mean_{b,m} min_n d2 )   (scalar fp32)

Strategy (v3 — shared-matrix, both directions per core):
  - 8 cores = 4 batches x 2 column-halves. Core (b, h) computes the NEGATED
    distance block  -d2  for all 4096 x-rows vs its 2048 y-columns, via a
    K=13 fp16 hi/lo-split augmented matmul (PSUM = 2xy - x^2 - y^2, fp32-
    accurate). Negation turns both reductions into MAX. This halves PE
    streaming (65536 cols/core vs 131072) and total drained elements vs
    computing the matrix twice (once per direction).
  - Per [128, 2048] PSUM tile: ACT casts it to fp16 (~1.9 us — the
    bottleneck: the cast is the only fp32->fp16 path out of PSUM, since
    DMA cannot read PSUM and Pool/GPSIMD has no max-capable ALU, so all
    max work lands on DVE at 2x fp16 throughput). DVE then (a) max-
    accumulates the col-max tile colA, (b) folds the row direction once
    (2048 -> 1024), whose strips DMA to the host for the final levels.
  - Pipeline details that matter: a tiny first DVE op reads PSUM+cast so
    the PSUM slot frees right after the cast (PE prefills 2 tiles ahead);
    a dummy activation preloads the ACT table during the input-DMA wait;
    total DMA instructions stay at 11 (the shared queue-slot sem pool) so
    no DMA ever needs a throttle wait on top of its data wait — walrus
    caps every instruction at ONE sync wait, which the _prune_redundant_
    waits pass enforces by transitive-implication analysis.
  - Host finish (order-independent): min over shipped strips + partition-
    axis min of colA, then mean / max. ~70.2 us vs the 131.7 us baseline.
"""

import numpy as np

B, N, M, D = 4, 4096, 4096, 3
KAUG = 13
P = 128            # partitions per row-tile
NCOL = M // 2      # 2048 columns per core
RT = N // P        # 32 row-tiles
SHIPW = 1024       # row-tree width shipped to the host per row-tile
SHIP_ENDS = (9, 17, 24, 29, 31)  # ship-group boundaries (row-tiles), big
                   # groups first so the queue is clear near the end; the
                   # last row-tile ships its raw cast instead.
NRAW = 1           # trailing row-tiles shipped as raw casts (no s1)
# Total DMA instructions are capped at 11 (the hardware's shared
# queue-slot semaphore pool): 4 inputs + 5 m1 ships + rawship + cola.
# A 12th DMA would reuse a slot sem and carry a throttle wait on top of
# its data wait, breaking walrus' one-sync-wait-per-instruction cap.
LO = np.float32(2.0 ** -11)  # power-of-2 pairing scale for the lo rows

_PROGRAM = None


def _build_program():
    import concourse.bass as bass
    import concourse.tile as tile
    from concourse import mybir

    f32 = mybir.dt.float32
    f16 = mybir.dt.float16
    MAX = mybir.AluOpType.max
    nc = bass.Bass(trn_type="TRN2")

    # One concatenated fp16 input: cols [0, N) lhs (stationary source),
    # cols [N, N+NCOL) rhs (moving).
    aug = nc.declare_dram_parameter("aug", [KAUG, N + NCOL], f16, isOutput=False)
    cola_d = nc.declare_dram_parameter("cola", [P, NCOL], f16, isOutput=True)
    # Per row-tile, a SHIPW-wide partial row-max strip; the host finishes
    # the last min-reduce levels (order-independent). The final row-tile
    # ships its raw cast instead (skipping its s1 shortens the kernel tail).
    ship_d = nc.declare_dram_parameter(
        "mship", [P, (RT - NRAW) * SHIPW], f16, isOutput=True
    )
    raw_d = nc.declare_dram_parameter(
        "rawship", [P, NRAW * NCOL], f16, isOutput=True
    )

    with tile.TileContext(nc) as tc:
        with (
            tc.tile_pool(name="singles", bufs=1) as singles,
            tc.tile_pool(name="psum", bufs=2, space="PSUM") as psum_pool,
            tc.tile_pool(name="cast", bufs=4) as cast_pool,
        ):
            aug_sb = singles.tile([KAUG, N + NCOL], f16)
            # All drain compute is on DVE: it is the only engine with a
            # max-capable ALU (walrus rejects max TensorTensor/scan on Pool;
            # ACT only casts). colA is the running col-max accumulator,
            # initialized by a plain copy on the first row-tile (no memset,
            # so no same-engine RAW sem is ever needed).
            colA = singles.tile([P, NCOL], f16)
            # m1all has one slice per row-tile — never reused, so the ship
            # DMAs impose no write-after-read waits on the tree.
            m1all = singles.tile([P, (RT - NRAW) * SHIPW], f16)
            dump = singles.tile([P, RT + 1], f16)
            # Dummy activation: loads the ACT function table during the
            # input-DMA wait instead of on the first real cast (~1.4 us).
            # Its operand column is memset first so nothing reads garbage.
            nc.vector.memset(dump[:, RT:], 0.0)
            nc.scalar.activation(
                dump[:, RT:], dump[:, RT:], mybir.ActivationFunctionType.Copy
            )
            # Four input DMAs, earliest-needed first: lhs head + rhs tail on
            # Pool, first rhs half on ACT (lowest queue latency), then the
            # lhs remainder (needed from row-tile 4, by when it has landed).
            nc.gpsimd.dma_start(out=aug_sb[:, :512], in_=aug[:, :512])
            nc.scalar.dma_start(out=aug_sb[:, N : N + 1024], in_=aug[:, N : N + 1024])
            nc.gpsimd.dma_start(out=aug_sb[:, N + 1024 :], in_=aug[:, N + 1024 :])
            nc.gpsimd.dma_start(out=aug_sb[:, 512:N], in_=aug[:, 512:N])
            rhs_sb = aug_sb[:, N:]

            def lhsT_of(rt):
                c = rt * P
                return aug_sb[:, c : c + P]

            H = NCOL // 2    # 1024: m1 width per unit (== SHIPW)
            ship_start = 0
            for rt in range(RT):
                lhsT = lhsT_of(rt)
                ps = psum_pool.tile([P, NCOL], f32)
                for q in range(NCOL // 512):  # one PSUM bank per matmul
                    nc.tensor.matmul(
                        ps[:, q * 512 : (q + 1) * 512],
                        lhsT,
                        rhs_sb[:, q * 512 : (q + 1) * 512],
                        start=True,
                        stop=True,
                    )
                # ACT: cast the whole tile to fp16 (enables DVE 2x mode).
                # ACT is the bottleneck engine: the cast is the only legal
                # fp32->fp16 path out of PSUM (DMA cannot read PSUM, Pool
                # has no max ALU), so ~0.83 ns/elem here is the kernel's
                # floor.
                cast16 = cast_pool.tile([P, NCOL], f16, tag="cast16")
                nc.scalar.activation(
                    cast16, ps, mybir.ActivationFunctionType.Copy
                )
                # Tiny PSUM+cast touch, FIRST DVE op of the unit: releases
                # the PSUM slot as soon as the cast is done (the PE can
                # prefill two tiles ahead instead of stalling behind the
                # whole DVE block), while still giving the next matmul's
                # slot-WAR wait a single DVE semaphore that transitively
                # implies the cast. One private dump column per row-tile:
                # no WAW chain.
                nc.vector.tensor_tensor(
                    out=dump[:, rt : rt + 1],
                    in0=ps[:, NCOL - 1 :],
                    in1=cast16[:, :1],
                    op=MAX,
                )
                # DVE block: col-max accumulate, then row-max tree level 1.
                if rt == 0:
                    nc.vector.tensor_copy(out=colA, in_=cast16)
                else:
                    nc.vector.tensor_tensor(
                        out=colA, in0=colA, in1=cast16, op=MAX
                    )
                if rt >= RT - NRAW:
                    # Trailing row-tiles: ship the raw cast directly — the
                    # DMA starts right after the cast instead of after s1,
                    # and the host does these tiles' pairing itself.
                    k = rt - (RT - NRAW)
                    nc.sync.dma_start(
                        out=raw_d[:, k * NCOL : (k + 1) * NCOL], in_=cast16
                    )
                else:
                    m1 = m1all[:, rt * H : (rt + 1) * H]
                    nc.vector.tensor_tensor(
                        out=m1,
                        in0=cast16[:, :H],
                        in1=cast16[:, H:],
                        op=MAX,
                    )
                    if rt + 1 in SHIP_ENDS:
                        # Ship this group's strips; the host does the final
                        # min levels (order-independent). The last groups
                        # are single tiles to keep the DMA tail short.
                        nc.sync.dma_start(
                            out=ship_d[:, ship_start * H : (rt + 1) * H],
                            in_=m1all[:, ship_start * H : (rt + 1) * H],
                        )
                        ship_start = rt + 1

            # cola goes out on the ACT queue's second slot (no throttle).
            nc.scalar.dma_start(out=cola_d[:], in_=colA)

    _dedupe_ldweights(nc)
    _prune_redundant_waits(nc)
    _split_multiwait_drains(nc)
    # No instruction may keep more than one sync wait (walrus cap).
    for fn in nc.m.functions:
        for blk in fn.blocks:
            for i in blk.instructions:
                si = getattr(i, "sync_info", None)
                assert si is None or len(si.on_wait) <= 1, (
                    f"{i.name} has {len(si.on_wait)} sync waits"
                )
    return nc


def _split_multiwait_drains(nc):
    """Walrus allows one sync wait per Drain: split a k-wait drain into a
    serial chain of single-wait drains on the same engine. The inserted
    drains update pre-registered sems so the race detector's fake-sem pass
    (which only sees framework-registered instructions) skips them."""
    from concourse import mybir

    sems = list(getattr(nc, "_drainsplit_sems", []))
    for fn in nc.m.functions:
        for blk in fn.blocks:
            out = []
            changed = False
            for i in blk.instructions:
                si = getattr(i, "sync_info", None)
                if (
                    type(i).__name__ == "InstDrain"
                    and si is not None
                    and len(si.on_wait) > 1
                ):
                    waits = list(si.on_wait)
                    for w in waits[:-1]:
                        d = mybir.InstDrain(
                            name=f"{i.name}-w{w.id}",
                            engine=i.engine,
                            ins=[],
                            outs=[],
                            bass_is_fusable=False,
                            sync_info=mybir.SyncInfo(
                                on_wait=[w], on_update=[]
                            ),
                        )
                        nc.register_instruction(d, overwrite=True)
                        out.append(d)
                    si.on_wait = [waits[-1]]
                    changed = True
                out.append(i)
            if changed:
                blk.instructions = out


def _dedupe_ldweights(nc):
    """Remove back-to-back identical Ldweights.

    The fp16 matmul lowering emits one standalone InstLdweights per matmul,
    but the PE array keeps the stationary operand until the next load — four
    matmuls sharing one lhsT only need the first load. A duplicate is removed
    only if its operand signature matches the previous kept Ldweights with no
    other Ldweights in between; its waits/updates (normally none) migrate to
    the next instruction.
    """
    for fn in nc.m.functions:
        for blk in fn.blocks:
            insts = list(blk.instructions)
            kept = []
            removed = 0
            last_sig = None
            pending = None  # sync carried from a removed LW
            for i in insts:
                if type(i).__name__ == "InstLdweights":
                    sig = (
                        str(i.ins[0]),
                        str(getattr(i, "tile_position", None)),
                        str(getattr(i, "tile_size", None)),
                        str(getattr(i, "perf_mode", None)),
                    )
                    if sig == last_sig:
                        si = i.sync_info
                        if si is not None and (si.on_wait or si.on_update):
                            pending = (
                                list(si.on_wait) + (pending[0] if pending else []),
                                list(si.on_update) + (pending[1] if pending else []),
                            )
                        removed += 1
                        continue
                    last_sig = sig
                if pending is not None:
                    si = i.sync_info
                    if si is not None:
                        si.on_wait = list(si.on_wait) + pending[0]
                        si.on_update = list(si.on_update) + pending[1]
                        pending = None
                kept.append(i)
            if removed:
                assert pending is None
                blk.instructions = kept


def _prune_redundant_waits(nc):
    """Drop semaphore waits that are transitively implied by other waits.

    Walrus caps the number of sync waits per instruction, but Tile's sem
    assigner is not transitively minimal across processors. A wait (S >= v)
    on instruction I is redundant if it is implied by I's same-engine
    predecessor's dispatch-time knowledge plus the completion-time knowledge
    of the providers of I's other (kept) waits.

    Conservative model:
      - same-engine successors inherit only the predecessor's dispatch-time
        knowledge (engines pipeline, so completion effects are not assumed);
      - a kept wait (S >= v) contributes the completion knowledge of the
        instruction whose cumulative increments of S first reach v (sem
        increments fire at completion, after that instruction's own waits
        held);
      - semaphores that ever receive a non-increment update (barrier sems)
        are excluded entirely.
    """
    ordered = []
    for fn in nc.m.functions:
        for blk in fn.blocks:
            ordered.extend(blk.instructions)
    insts = [
        i
        for i in ordered
        if getattr(i, "sync_info", None) is not None
        and getattr(i, "engine", None) is not None
    ]

    bad_sems = set()

    def merge(dst, src):
        for s, v in src.items():
            if dst.get(s, -1) < v:
                dst[s] = v

    def implies(know, sem, val):
        return know.get(sem, -1) >= val

    sem_cum = {}        # sem id -> cumulative inc count so far
    sem_events = {}     # sem id -> list of (cum_after, inst_index)
    k_exec = []         # dispatch-time knowledge per inst index
    k_complete = []     # completion-time knowledge per inst index

    def provider(sem, val):
        for cum, idx in sem_events.get(sem, ()):
            if cum >= val:
                return idx
        return None

    sem_owner = {}
    for i in insts:
        for u in i.sync_info.on_update:
            sem_owner.setdefault(u.id, i.engine)
    engine_pos = {}
    engine_pos_of = {}

    # Pass 1: build the full knowledge tables (no modification). The block
    # instruction list interleaves engine streams in an arbitrary merged
    # order, so an instruction may legitimately wait on semaphore values
    # provided "later" in the list — the tables must be complete before
    # pruning. Knowledge from waits that pass 2 removes is identical (they
    # are implied), so pass-1 tables remain valid.
    last_on_proc = {}
    for n, i in enumerate(insts):
        si = i.sync_info
        my_pos = engine_pos.get(i.engine, 0)
        prev = last_on_proc.get(i.engine)
        base = dict(k_exec[prev]) if prev is not None else {}
        ke = dict(base)
        for w in si.on_wait:
            if w.wait_mode == "sem-ge-imm" and w.id not in bad_sems:
                know = {w.id: w.wait_value}
                p = provider(w.id, w.wait_value)
                if p is not None and p < n:
                    merge(know, k_complete[p])
                merge(ke, know)
        kc = dict(ke)
        for u in si.on_update:
            if u.update_mode not in ("sem-inc", "sem-add-imm") or u.update_value <= 0:
                bad_sems.add(u.id)
            elif u.id not in bad_sems:
                cum = sem_cum.get(u.id, 0) + u.update_value
                sem_cum[u.id] = cum
                sem_events.setdefault(u.id, []).append((cum, n))
                if kc.get(u.id, -1) < cum:
                    kc[u.id] = cum
        # DMA waits gate the DMA queue, not the issuing engine: the engine's
        # next instruction must not inherit wait-derived knowledge from a DMA.
        # Updates (kc) are NOT inherited by same-engine successors: engines
        # pipeline their memory acks, so a same-engine RAW still needs the
        # sem-valued wait.
        k_exec.append(base if "DMA" in type(i).__name__ else ke)
        k_complete.append(kc)
        last_on_proc[i.engine] = n
        engine_pos_of[n] = my_pos
        engine_pos[i.engine] = my_pos + 1

    # Pass 1 above left provider-knowledge incomplete for forward references
    # (p >= n). Iterate once more to a fixpoint-ish refinement: recompute
    # ke/kc with the full event table. Two sweeps suffice for the chains we
    # prune (provider chains are short).
    for _sweep in range(2):
        last_on_proc = {}
        for n, i in enumerate(insts):
            si = i.sync_info
            prev = last_on_proc.get(i.engine)
            base = dict(k_exec[prev]) if prev is not None else {}
            ke = dict(base)
            for w in si.on_wait:
                if w.wait_mode == "sem-ge-imm" and w.id not in bad_sems:
                    know = {w.id: w.wait_value}
                    p = provider(w.id, w.wait_value)
                    if p is not None and p != n:
                        merge(know, k_complete[p])
                    merge(ke, know)
            kc = dict(ke)
            for u in si.on_update:
                if u.update_mode in ("sem-inc", "sem-add-imm") and u.id not in bad_sems:
                    for cum, idx in sem_events.get(u.id, ()):
                        if idx == n and kc.get(u.id, -1) < cum:
                            kc[u.id] = cum
            k_exec[n] = base if "DMA" in type(i).__name__ else ke
            k_complete[n] = kc
            last_on_proc[i.engine] = n

    # Pass 2: prune with the complete tables.
    last_on_proc = {}
    for n, i in enumerate(insts):
        si = i.sync_info
        waits = list(si.on_wait)
        my_pos = engine_pos_of[n]

        # Drop a wait on the instruction's own engine's semaphore when the
        # providing instruction is >= 2 same-engine instructions back AND
        # the wait is not a read-after-write (CoreSim's race detector
        # requires a semaphore observation for RAW once the writer carries a
        # sem update; WAR/WAW ride the engine's serial execution).
        def _memrefs(args):
            names = set()
            for a in args:
                mr = getattr(a, "memref", None)
                if mr is None:
                    t = getattr(a, "tensor", None)
                    mr = getattr(t, "name", None)
                if mr is not None:
                    names.add(str(mr))
            return names

        if len(waits) > 1:
            my_reads = _memrefs(getattr(i, "ins", []) or [])
            kept0 = []
            for w in waits:
                if (
                    w.wait_mode == "sem-ge-imm"
                    and w.id not in bad_sems
                    and sem_owner.get(w.id) == i.engine
                ):
                    p = provider(w.id, w.wait_value)
                    if p is not None and p in engine_pos_of:
                        p_writes = _memrefs(getattr(insts[p], "outs", []) or [])
                        if my_pos - engine_pos_of[p] >= 2 and not (
                            my_reads & p_writes
                        ):
                            continue
                kept0.append(w)
            if len(kept0) < len(waits):
                si.on_wait = kept0
                waits = kept0

        prunable = (
            len(waits) > 1
            and all(w.wait_mode == "sem-ge-imm" and w.id not in bad_sems for w in waits)
        )

        prev = last_on_proc.get(i.engine)
        base = dict(k_exec[prev]) if prev is not None else {}

        def wait_know(w):
            know = {w.id: w.wait_value}
            p = provider(w.id, w.wait_value)
            if p is not None and p != n:
                merge(know, k_complete[p])
            return know

        if prunable:
            kept = None
            # try to cover everything with a single wait
            for cand in reversed(waits):
                know = dict(base)
                merge(know, wait_know(cand))
                if all(
                    w is cand or implies(know, w.id, w.wait_value) for w in waits
                ):
                    kept = [cand]
                    break
            if kept is None:
                # strengthen: wait LONGER on one sem if some provider's
                # completion knowledge implies every other wait (sound: a
                # higher wait value only delays this instruction). Only
                # cross-engine providers are eligible — a same-engine
                # provider later in the stream would deadlock it.
                for cand in waits:
                    if kept is not None:
                        break
                    for cum, idx in sem_events.get(cand.id, ()):
                        if cum < cand.wait_value or idx == n:
                            continue
                        if insts[idx].engine == i.engine:
                            continue
                        know = dict(base)
                        know[cand.id] = cum
                        merge(know, k_complete[idx])
                        if all(
                            w is cand or implies(know, w.id, w.wait_value)
                            for w in waits
                        ):
                            cand.wait_value = cum
                            kept = [cand]
                            break
            if kept is None:
                # greedy: add waits until all are covered
                kept = []
                know = dict(base)
                for cand in reversed(waits):
                    if not implies(know, cand.id, cand.wait_value):
                        kept.append(cand)
                        merge(know, wait_know(cand))
            if len(kept) < len(waits):
                si.on_wait = kept
                waits = kept

        last_on_proc[i.engine] = n


def _get_program():
    global _PROGRAM
    if _PROGRAM is None:
        _PROGRAM = _build_program()
    return _PROGRAM


def _split16(v):
    """Exact fp16 hi/lo split: v ~= hi + lo16 * 2^-11 with ~2^-24 residual."""
    hi = v.astype(np.float16)
    lo32 = v - hi.astype(np.float32)
    lo16 = (lo32 * np.float32(2048.0)).astype(np.float16)
    return hi, lo16


def _augment(R, C):
    """K=13 fp16 hi/lo-split augmented operands, NEGATED distances.

    PSUM accumulates -d2[n, m] = 2 R_n.C_m - |R_n|^2 - |C_m|^2 in fp32 with
    ~1e-6 absolute error: every hi*hi, hi*lo, lo*hi product is kept (fp16
    products are exact in fp32); lo rows carry a 2^11 scale paired with
    2^-11 on the opposite side so nothing lands in fp16 subnormals.
    """
    nr, mc = R.shape[0], C.shape[0]
    lhs = np.empty((KAUG, nr), np.float16)
    rhs = np.empty((KAUG, mc), np.float16)
    a = 2.0 * R.T.astype(np.float32)   # +2 for the negated matrix
    y = C.T.astype(np.float32)
    a_hi, a_lo = _split16(a)
    y_hi, y_lo = _split16(y)
    lhs[0:3] = a_hi
    rhs[0:3] = y_hi
    lhs[3:6] = (a_hi.astype(np.float32) * LO).astype(np.float16)
    rhs[3:6] = y_lo
    lhs[6:9] = a_lo
    rhs[6:9] = (y_hi.astype(np.float32) * LO).astype(np.float16)
    x2_hi, x2_lo = _split16(np.sum(R.astype(np.float32) ** 2, axis=1))
    y2_hi, y2_lo = _split16(np.sum(C.astype(np.float32) ** 2, axis=1))
    lhs[9] = -x2_hi
    rhs[9] = 1.0
    lhs[10] = -x2_lo
    rhs[10] = LO
    lhs[11] = -1.0
    rhs[11] = y2_hi
    lhs[12] = -LO
    rhs[12] = y2_lo
    return lhs, rhs


def make_in_maps(x, y):
    x = np.asarray(x, dtype=np.float32)
    y = np.asarray(y, dtype=np.float32)
    in_maps = []
    for c in range(8):
        b, h = c // 2, c % 2
        R = x[b]
        C = y[b][h * NCOL : (h + 1) * NCOL]
        lhs, rhs = _augment(R, C)
        in_maps.append({"aug": np.concatenate([lhs, rhs], axis=1)})
    return in_maps


def combine(results):
    """Finish the reductions on the host.

    Per core (b, h):
      mship [128, RT*SHIPW] fp16: mship[p, rt*SHIPW + j] = partial max of -d2
        for x-point n = 128*rt + p over its y-column group j (partial row
        min; reduce over j, then merge h=0/1).
      cola [128, 2048] fp16: column accumulator; max over partitions gives
        the exact per-y-point max of -d2.
    """
    row_negmax = []  # per core: [4096] partial max of -d2
    col_mins = []    # per-y-point min d2 (exact), all cores
    for c in range(8):
        r = results[c]
        ms = np.asarray(r["mship"]).reshape(P, RT - NRAW, SHIPW)
        raw = np.asarray(r["rawship"]).reshape(P, NRAW, NCOL)
        rp = np.empty((P, RT), np.float32)
        rp[:, : RT - NRAW] = ms.max(axis=2)
        rp[:, RT - NRAW :] = raw.max(axis=2)
        row_negmax.append(rp.T.ravel())                   # x-point n=128*rt+p
        ca = np.asarray(r["cola"], dtype=np.float32).max(axis=0)
        col_mins.append(np.maximum(-ca, 0.0))
    x_mins = []
    for b in range(4):
        m = np.maximum(row_negmax[2 * b], row_negmax[2 * b + 1])
        x_mins.append(np.maximum(-m, 0.0))
    x_to_y = np.concatenate(x_mins).astype(np.float64).mean()
    y_to_x = np.concatenate(col_mins).astype(np.float64).mean()
    return np.array(max(x_to_y, y_to_x), dtype=np.float32)


def kernel(x, y):
    from concourse.bass_utils import run_bass_kernel_spmd

    nc = _get_program()
    in_maps = make_in_maps(x, y)
    res = run_bass_kernel_spmd(nc, in_maps, list(range(8)))
    return combine(res.results)


if __name__ == "__main__":
    xs = np.random.randn(B, N, D).astype(np.float32)
    ys = np.random.randn(B, M, D).astype(np.float32)
    print(kernel(xs, ys))
